# revision 9
# baseline (speedup 1.0000x reference)
"""DeepInsight encoding kernel for 8 Trainium2 NeuronCores.

Data-parallel over batch: each core builds 64 interleaved [H, W*5] output
planes in SBUF and streams them to HBM as large contiguous DMAs.

Channels per output plane [h, w, c]:
  c0: stamp (static, written once per rotating buffer)
  c1: scatter-add of x at coords (PE matmul with host-built one-hots)
  c2: row-wise copy x[row_idx[h]] (PE matmul, broadcast along w)
  c3: |x_i - x_j| / (max-min) upsampled 4x4 (DVE subtract+abs_max)
  c4: equidistant bars y < round(128*x) (DVE is_gt vs iota), gaps static 0
"""

import numpy as np

B, D, H, W, C = 512, 32, 128, 128, 5
NCORES = 8
BPC = B // NCORES            # 64 batches per core
G = 8                        # batches per output DMA group
NGROUPS = BPC // G           # 8
NBUF = 3                     # rotating SBUF plane buffers
FP = W * C                   # 640 floats per output row
MAGIC = float(2.0 ** 23)     # fp32 round-to-nearest-even trick

_RUNNER = None


def _build_nc():
    import concourse.bacc as bacc
    import concourse.mybir as mybir
    from concourse.tile import TileContext

    f32 = mybir.dt.float32
    alu = mybir.AluOpType
    act = mybir.ActivationFunctionType

    # Bacc (not raw Bass): its finalize() pipeline runs
    # move_matmul_waits_to_ldweights + generate_event_semaphores, which
    # split sync-waits to satisfy TRN2's 1-wait-per-instruction limit.
    nc = bacc.Bacc()
    x_bm_d = nc.dram_tensor("x_bm", [BPC, D], f32, kind="ExternalInput")
    x_t_d = nc.dram_tensor("x_t", [D, BPC], f32, kind="ExternalInput")
    stamp_d = nc.dram_tensor("stamp2d", [H, W], f32, kind="ExternalInput")
    # consts32 = [scatR | scatC | onehotR], each [D, 128]
    consts_d = nc.dram_tensor("consts32", [D, 3 * W], f32, kind="ExternalInput")
    ident_d = nc.dram_tensor("ident64", [BPC, BPC], f32, kind="ExternalInput")
    # misc: col0 = iota(0..127), col1 = ones
    misc_d = nc.dram_tensor("misc", [H, 2], f32, kind="ExternalInput")
    out_d = nc.dram_tensor("out", [BPC, H, FP], f32, kind="ExternalOutput")

    with TileContext(nc) as tc:
        with (
            tc.tile_pool(name="const", bufs=1) as cpool,
            tc.tile_pool(name="gbuf", bufs=1) as gpool,
            tc.tile_pool(name="work", bufs=4) as wpool,
            tc.tile_pool(name="small", bufs=1) as spool,
        ):
            # ---- load constants / inputs
            x_bm = cpool.tile([BPC, D], f32, tag="x_bm")
            x_t = cpool.tile([D, BPC], f32, tag="x_t")
            stamp = cpool.tile([H, W], f32, tag="stamp")
            consts = cpool.tile([D, 3 * W], f32, tag="consts")
            ident = cpool.tile([BPC, BPC], f32, tag="ident")
            misc = cpool.tile([H, 2], f32, tag="misc")
            nc.sync.dma_start(out=x_bm[:, :], in_=x_bm_d[:, :])
            nc.sync.dma_start(out=x_t[:, :], in_=x_t_d[:, :])
            nc.sync.dma_start(out=stamp[:, :], in_=stamp_d[:, :])
            nc.sync.dma_start(out=consts[:, :], in_=consts_d[:, :])
            nc.sync.dma_start(out=ident[:, :], in_=ident_d[:, :])
            nc.sync.dma_start(out=misc[:, :], in_=misc_d[:, :])
            scatR = consts[:, 0:W]
            scatC = consts[:, W : 2 * W]
            onehotR = consts[:, 2 * W : 3 * W]
            iota = misc[:, 0:1]

            # ---- setup: invr, scaled x, bar heights, row-gathered xs
            inv_sb = spool.tile([BPC, 4], f32, tag="inv")  # r|min|max|invr cols
            xbh = spool.tile([BPC, 3 * D], f32, tag="xbh")  # [xs_bm | bh | -xs_bm]
            xs_t = spool.tile([D, BPC], f32, tag="xs_t")
            xrs = spool.tile([H, BPC], f32, tag="xrs")
            negxrs = spool.tile([H, BPC], f32, tag="negxrs")
            invr_row = spool.tile([1, BPC], f32, tag="invr_row")

            nc.vector.tensor_reduce(
                inv_sb[:, 1:2], x_bm[:, :], axis=mybir.AxisListType.X, op=alu.min
            )
            nc.vector.tensor_reduce(
                inv_sb[:, 2:3], x_bm[:, :], axis=mybir.AxisListType.X, op=alu.max
            )
            # r = max - min
            nc.vector.tensor_tensor(
                out=inv_sb[:, 0:1],
                in0=inv_sb[:, 2:3],
                in1=inv_sb[:, 1:2],
                op=alu.subtract,
            )
            nc.vector.reciprocal(inv_sb[:, 3:4], inv_sb[:, 0:1])

            # xs_bm = x_bm * invr (per-partition scalar)
            nc.vector.tensor_scalar(
                out=xbh[:, 0:D],
                in0=x_bm[:, :],
                scalar1=inv_sb[:, 3:4],
                scalar2=None,
                op0=alu.mult,
            )
            # -xs for the |a-b| = max(relu(d), -d) two-op trick
            nc.vector.tensor_scalar(
                out=xbh[:, 2 * D : 3 * D],
                in0=xbh[:, 0:D],
                scalar1=-1.0,
                scalar2=None,
                op0=alu.mult,
            )
            # bh = round_half_even(128 * x): (x*128 + 2^23) - 2^23
            nc.vector.tensor_scalar(
                out=xbh[:, D : 2 * D],
                in0=x_bm[:, :],
                scalar1=128.0,
                scalar2=MAGIC,
                op0=alu.mult,
                op1=alu.add,
            )
            nc.vector.tensor_scalar(
                out=xbh[:, D : 2 * D],
                in0=xbh[:, D : 2 * D],
                scalar1=MAGIC,
                scalar2=None,
                op0=alu.subtract,
            )

            with tc.tile_pool(name="psetup", bufs=2, space="PSUM") as psetup:
                # invr_row [1, 64] = transpose(invr_col) via PE
                invr_row_ps = psetup.tile([1, BPC], f32, tag="invr_row_ps")
                nc.tensor.transpose(
                    invr_row_ps[:, :], inv_sb[:, 3:4], ident[:, :]
                )
                nc.vector.tensor_copy(invr_row[:, :], invr_row_ps[:, :])

                # invr_bc [32, 64]: broadcast invr_row to 32 partitions
                # (K=1 matmul with ones lhsT from misc col1)
                invr_bc_ps = psetup.tile([D, BPC], f32, tag="invr_bc_ps")
                nc.tensor.matmul(
                    invr_bc_ps[:, :],
                    misc[0:1, 1:2].broadcast_to([1, D]),
                    invr_row[:, :],
                )
                # xs_t = x_t * invr_bc
                nc.vector.tensor_tensor(
                    out=xs_t[:, :], in0=x_t[:, :], in1=invr_bc_ps[:, :], op=alu.mult
                )
                # xrs [128, 64] = onehotR.T @ xs_t  (xs at row_idx[h], per batch col)
                xrs_ps = psetup.tile([H, BPC], f32, tag="xrs_ps")
                nc.tensor.matmul(xrs_ps[:, :], onehotR, xs_t[:, :])
                nc.vector.tensor_copy(xrs[:, :], xrs_ps[:, :])
                nc.vector.tensor_scalar(
                    out=negxrs[:, :],
                    in0=xrs_ps[:, :],
                    scalar1=-1.0,
                    scalar2=None,
                    op0=alu.mult,
                )

            # ---- rotating group buffers with static channel content
            gbufs = []
            for k in range(NBUF):
                buf = gpool.tile([H, G * FP], f32, tag=f"gbuf{k}")
                v4 = buf[:, :].rearrange("p (g w c) -> p g w c", g=G, c=C)
                # c0 = stamp for every group slot
                nc.gpsimd.tensor_copy(
                    v4[:, :, :, 0],
                    stamp[:, None, :].broadcast_to([H, G, W]),
                )
                # c4 gaps = 0 (bars live at w = 17 + 3i)
                nc.gpsimd.memset(v4[:, :, 0:17, 4], 0.0)
                nc.gpsimd.memset(v4[:, :, 18:110:3, 4], 0.0)
                nc.gpsimd.memset(v4[:, :, 19:111:3, 4], 0.0)
                nc.gpsimd.memset(v4[:, :, 111:128, 4], 0.0)
                gbufs.append(buf)

            with (
                tc.tile_pool(name="p12", bufs=4, space="PSUM") as p12pool,
                tc.tile_pool(name="pbc", bufs=4, space="PSUM") as pbcpool,
            ):
                for g in range(NGROUPS):
                    buf = gbufs[g % NBUF]
                    for j in range(G):
                        b = g * G + j
                        plane = buf[:, j * FP : (j + 1) * FP]
                        vc = plane.rearrange("p (w c) -> p w c", c=C)

                        # scratch2 = scatC * x[b, :] (per-partition scalar)
                        scr = wpool.tile([D, W], f32, tag="scr")
                        nc.gpsimd.tensor_scalar(
                            out=scr[:, :],
                            in0=scatC,
                            scalar1=x_t[:, b : b + 1],
                            scalar2=None,
                            op0=alu.mult,
                        )
                        # c1 scatter + c2 rowcopy into one PSUM tile
                        p12 = p12pool.tile([H, 2 * W], f32, tag="p12")
                        nc.tensor.matmul(p12[:, 0:W], scatR, scr[:, :])
                        nc.tensor.matmul(
                            p12[:, W : 2 * W],
                            onehotR,
                            x_t[:, b : b + 1].broadcast_to([D, W]),
                        )
                        # partition-broadcast of [xs|bh|-xs] row b via basis col
                        pbc = pbcpool.tile([H, 3 * D], f32, tag="pbc")
                        nc.tensor.matmul(
                            pbc[:, :],
                            ident[:, b : b + 1].broadcast_to([BPC, H]),
                            xbh[:, :],
                        )
                        # c3 = |xs_col - xs_row| as max(relu(d), -d)
                        # (abs_max is not encodable on this DVE table)
                        c3v = vc[:, :, 3].rearrange("p (d r) -> p d r", r=4)
                        nc.vector.tensor_scalar(
                            out=c3v,
                            in0=pbc[:, 0:D].unsqueeze(2).broadcast_to([H, D, 4]),
                            scalar1=xrs[:, b : b + 1],
                            scalar2=0.0,
                            op0=alu.subtract,
                            op1=alu.max,
                        )
                        nc.vector.scalar_tensor_tensor(
                            out=c3v,
                            in0=pbc[:, 2 * D : 3 * D]
                            .unsqueeze(2)
                            .broadcast_to([H, D, 4]),
                            scalar=negxrs[:, b : b + 1],
                            in1=c3v,
                            op0=alu.subtract,
                            op1=alu.max,
                        )
                        # c4 bars = (bh > iota)
                        nc.vector.tensor_scalar(
                            out=vc[:, 17:111:3, 4],
                            in0=pbc[:, D : 2 * D],
                            scalar1=iota,
                            scalar2=None,
                            op0=alu.is_gt,
                        )
                        # c1, c2 <- p12 (interleave pairs)
                        nc.scalar.activation(
                            vc[:, :, 1:3],
                            p12[:, :].rearrange("p (c w) -> p w c", c=2),
                            act.Copy,
                        )
                    nc.sync.dma_start(
                        out=out_d[g * G : (g + 1) * G, :, :].rearrange(
                            "b h f -> h b f"
                        ),
                        in_=buf[:, :].rearrange("p (g f) -> p g f", g=G),
                    )
    nc.finalize()
    return nc


def _host_inputs(inputs, stamp, coords):
    """Build the 8 per-core input maps."""
    x = np.ascontiguousarray(inputs, dtype=np.float32)
    stamp2d = np.ascontiguousarray(stamp.reshape(H, W), dtype=np.float32)
    coords = np.asarray(coords)

    scatR = np.zeros((D, W), np.float32)
    scatC = np.zeros((D, W), np.float32)
    scatR[np.arange(D), coords[:, 0]] = 1.0
    scatC[np.arange(D), coords[:, 1]] = 1.0
    # onehotR[d, h] = 1 where row_idx[h] == d
    row_idx = np.repeat(np.arange(D), H // D)
    onehotR = np.zeros((D, H), np.float32)
    onehotR[row_idx, np.arange(H)] = 1.0
    consts32 = np.ascontiguousarray(
        np.concatenate([scatR, scatC, onehotR], axis=1), np.float32
    )
    ident64 = np.eye(BPC, dtype=np.float32)
    misc = np.zeros((H, 2), np.float32)
    misc[:, 0] = np.arange(H)
    misc[:, 1] = 1.0

    maps = []
    for m in range(NCORES):
        xs = x[m * BPC : (m + 1) * BPC]
        maps.append(
            {
                "x_bm": np.ascontiguousarray(xs),
                "x_t": np.ascontiguousarray(xs.T),
                "stamp2d": stamp2d,
                "consts32": consts32,
                "ident64": ident64,
                "misc": misc,
            }
        )
    return maps


class _Runner:
    """Builds the Bass program once and caches the jitted SPMD executable."""

    def __init__(self):
        self.nc = _build_nc()
        self._sharded = None
        self._meta = None

    def _build_exec(self):
        import jax
        import numpy as np
        import concourse.mybir as mybir
        from concourse import bass2jax
        from jax.sharding import Mesh, PartitionSpec
        from jax.experimental.shard_map import shard_map

        bass2jax.install_neuronx_cc_hook()
        nc = self.nc
        partition_name = (
            nc.partition_id_tensor.name if nc.partition_id_tensor else None
        )
        in_names, out_names, out_avals, zero_shapes = [], [], [], []
        for alloc in nc.m.functions[0].allocations:
            if not isinstance(alloc, mybir.MemoryLocationSet):
                continue
            name = alloc.memorylocations[0].name
            if alloc.kind == "ExternalInput":
                if name != partition_name:
                    in_names.append(name)
            elif alloc.kind == "ExternalOutput":
                shape = tuple(alloc.tensor_shape)
                dtype = mybir.dt.np(alloc.dtype)
                out_names.append(name)
                out_avals.append(jax.core.ShapedArray(shape, dtype))
                zero_shapes.append((shape, dtype))
        n_params = len(in_names)
        all_names = in_names + out_names
        if partition_name is not None:
            all_names = all_names + [partition_name]
        donate = tuple(range(n_params, n_params + len(out_names)))

        def _body(*args):
            operands = list(args)
            if partition_name is not None:
                operands.append(bass2jax.partition_id_tensor())
            outs = bass2jax._bass_exec_p.bind(
                *operands,
                out_avals=tuple(out_avals),
                in_names=tuple(all_names),
                out_names=tuple(out_names),
                lowering_input_output_aliases=(),
                sim_require_finite=True,
                sim_require_nnan=True,
                nc=nc,
            )
            return tuple(outs)

        devices = jax.devices()[:NCORES]
        mesh = Mesh(np.asarray(devices), ("core",))
        in_specs = (PartitionSpec("core"),) * (n_params + len(out_names))
        out_specs = (PartitionSpec("core"),) * len(out_names)
        sharded = jax.jit(
            shard_map(
                _body,
                mesh=mesh,
                in_specs=in_specs,
                out_specs=out_specs,
                check_rep=False,
            ),
            donate_argnums=donate,
            keep_unused=True,
        )
        self._sharded = sharded
        self._meta = (in_names, out_names, zero_shapes)

    def run(self, in_maps):
        if self._sharded is None:
            self._build_exec()
        in_names, out_names, zero_shapes = self._meta
        concat_in = [
            np.concatenate([np.asarray(m[name]) for m in in_maps], axis=0)
            for name in in_names
        ]
        concat_zeros = [
            np.zeros((NCORES * s[0], *s[1:]), dt) for (s, dt) in zero_shapes
        ]
        out_arrs = self._sharded(*concat_in, *concat_zeros)
        outs = [np.asarray(a) for a in out_arrs]
        per_core = []
        for c in range(NCORES):
            per_core.append(
                {
                    name: outs[i].reshape(NCORES, *zero_shapes[i][0])[c]
                    for i, name in enumerate(out_names)
                }
            )
        return per_core


def _get_runner():
    global _RUNNER
    if _RUNNER is None:
        _RUNNER = _Runner()
    return _RUNNER


def kernel(inputs, stamp, coords):
    inputs = np.asarray(inputs)
    stamp = np.asarray(stamp)
    coords = np.asarray(coords)
    runner = _get_runner()
    in_maps = _host_inputs(inputs, stamp, coords)
    results = runner.run(in_maps)
    out = np.stack([r["out"] for r in results], axis=0)  # [8, 64, H, W*C]
    out = out.reshape(B, H, W, C).astype(np.float32)
    return out


# revision 11
# speedup vs baseline: 26.1903x; 26.1903x over previous
"""DeepInsight encoding kernel for 8 Trainium2 NeuronCores.

Data-parallel over batch: each core builds 64 interleaved [H, W*5] output
planes in SBUF and streams them to HBM as large contiguous DMAs.

Channels per output plane [h, w, c]:
  c0: stamp (static, written once per rotating buffer)
  c1: scatter-add of x at coords (PE matmul with host-built one-hots)
  c2: row-wise copy x[row_idx[h]] (PE matmul, broadcast along w)
  c3: |x_i - x_j| / (max-min) upsampled 4x4 (DVE subtract+abs_max)
  c4: equidistant bars y < round(128*x) (DVE is_gt vs iota), gaps static 0
"""

import numpy as np

B, D, H, W, C = 512, 32, 128, 128, 5
NCORES = 8
BPC = B // NCORES            # 64 batches per core
G = 8                        # batches per output DMA group
NGROUPS = BPC // G           # 8
NBUF = 3                     # rotating SBUF plane buffers
FP = W * C                   # 640 floats per output row
MAGIC = float(2.0 ** 23)     # fp32 round-to-nearest-even trick

_RUNNER = None


def _build_nc():
    import concourse.bacc as bacc
    import concourse.mybir as mybir
    from concourse.tile import TileContext

    f32 = mybir.dt.float32
    alu = mybir.AluOpType
    act = mybir.ActivationFunctionType

    # Bacc (not raw Bass): its finalize() pipeline runs
    # move_matmul_waits_to_ldweights + generate_event_semaphores, which
    # split sync-waits to satisfy TRN2's 1-wait-per-instruction limit.
    nc = bacc.Bacc()
    x_bm_d = nc.dram_tensor("x_bm", [BPC, D], f32, kind="ExternalInput")
    x_t_d = nc.dram_tensor("x_t", [D, BPC], f32, kind="ExternalInput")
    stamp_d = nc.dram_tensor("stamp2d", [H, W], f32, kind="ExternalInput")
    # consts32 = [scatR | scatC | onehotR], each [D, 128]
    consts_d = nc.dram_tensor("consts32", [D, 3 * W], f32, kind="ExternalInput")
    ident_d = nc.dram_tensor("ident64", [BPC, BPC], f32, kind="ExternalInput")
    # misc: col0 = iota(0..127), col1 = ones
    misc_d = nc.dram_tensor("misc", [H, 2], f32, kind="ExternalInput")
    out_d = nc.dram_tensor("out", [BPC, H, FP], f32, kind="ExternalOutput")

    with TileContext(nc) as tc:
        with (
            tc.tile_pool(name="const", bufs=1) as cpool,
            tc.tile_pool(name="gbuf", bufs=1) as gpool,
            tc.tile_pool(name="work", bufs=4) as wpool,
            tc.tile_pool(name="small", bufs=1) as spool,
        ):
            # ---- load constants / inputs
            x_bm = cpool.tile([BPC, D], f32, tag="x_bm")
            x_t = cpool.tile([D, BPC], f32, tag="x_t")
            stamp = cpool.tile([H, W], f32, tag="stamp")
            consts = cpool.tile([D, 3 * W], f32, tag="consts")
            ident = cpool.tile([BPC, BPC], f32, tag="ident")
            misc = cpool.tile([H, 2], f32, tag="misc")
            nc.sync.dma_start(out=x_bm[:, :], in_=x_bm_d[:, :])
            nc.sync.dma_start(out=x_t[:, :], in_=x_t_d[:, :])
            nc.sync.dma_start(out=stamp[:, :], in_=stamp_d[:, :])
            nc.sync.dma_start(out=consts[:, :], in_=consts_d[:, :])
            nc.sync.dma_start(out=ident[:, :], in_=ident_d[:, :])
            nc.sync.dma_start(out=misc[:, :], in_=misc_d[:, :])
            scatR = consts[:, 0:W]
            scatC = consts[:, W : 2 * W]
            onehotR = consts[:, 2 * W : 3 * W]
            iota = misc[:, 0:1]

            # ---- setup: invr, scaled x, bar heights, row-gathered xs
            inv_sb = spool.tile([BPC, 4], f32, tag="inv")  # r|min|max|invr cols
            xbh = spool.tile([BPC, 3 * D], f32, tag="xbh")  # [xs_bm | bh | -xs_bm]
            xs_t = spool.tile([D, BPC], f32, tag="xs_t")
            xrs = spool.tile([H, BPC], f32, tag="xrs")
            negxrs = spool.tile([H, BPC], f32, tag="negxrs")
            invr_row = spool.tile([1, BPC], f32, tag="invr_row")

            nc.vector.tensor_reduce(
                inv_sb[:, 1:2], x_bm[:, :], axis=mybir.AxisListType.X, op=alu.min
            )
            nc.vector.tensor_reduce(
                inv_sb[:, 2:3], x_bm[:, :], axis=mybir.AxisListType.X, op=alu.max
            )
            # r = max - min
            nc.vector.tensor_tensor(
                out=inv_sb[:, 0:1],
                in0=inv_sb[:, 2:3],
                in1=inv_sb[:, 1:2],
                op=alu.subtract,
            )
            nc.vector.reciprocal(inv_sb[:, 3:4], inv_sb[:, 0:1])

            # xs_bm = x_bm * invr (per-partition scalar)
            nc.vector.tensor_scalar(
                out=xbh[:, 0:D],
                in0=x_bm[:, :],
                scalar1=inv_sb[:, 3:4],
                scalar2=None,
                op0=alu.mult,
            )
            # -xs for the |a-b| = max(relu(d), -d) two-op trick
            nc.vector.tensor_scalar(
                out=xbh[:, 2 * D : 3 * D],
                in0=xbh[:, 0:D],
                scalar1=-1.0,
                scalar2=None,
                op0=alu.mult,
            )
            # bh = round_half_even(128 * x): (x*128 + 2^23) - 2^23
            nc.vector.tensor_scalar(
                out=xbh[:, D : 2 * D],
                in0=x_bm[:, :],
                scalar1=128.0,
                scalar2=MAGIC,
                op0=alu.mult,
                op1=alu.add,
            )
            nc.vector.tensor_scalar(
                out=xbh[:, D : 2 * D],
                in0=xbh[:, D : 2 * D],
                scalar1=MAGIC,
                scalar2=None,
                op0=alu.subtract,
            )

            with tc.tile_pool(name="psetup", bufs=2, space="PSUM") as psetup:
                # invr_row [1, 64] = transpose(invr_col) via PE
                invr_row_ps = psetup.tile([1, BPC], f32, tag="invr_row_ps")
                nc.tensor.transpose(
                    invr_row_ps[:, :], inv_sb[:, 3:4], ident[:, :]
                )
                nc.vector.tensor_copy(invr_row[:, :], invr_row_ps[:, :])

                # invr_bc [32, 64]: broadcast invr_row to 32 partitions
                # (K=1 matmul with ones lhsT from misc col1)
                invr_bc_ps = psetup.tile([D, BPC], f32, tag="invr_bc_ps")
                nc.tensor.matmul(
                    invr_bc_ps[:, :],
                    misc[0:1, 1:2].broadcast_to([1, D]),
                    invr_row[:, :],
                )
                # xs_t = x_t * invr_bc
                nc.vector.tensor_tensor(
                    out=xs_t[:, :], in0=x_t[:, :], in1=invr_bc_ps[:, :], op=alu.mult
                )
                # xrs [128, 64] = onehotR.T @ xs_t  (xs at row_idx[h], per batch col)
                xrs_ps = psetup.tile([H, BPC], f32, tag="xrs_ps")
                nc.tensor.matmul(xrs_ps[:, :], onehotR, xs_t[:, :])
                nc.vector.tensor_copy(xrs[:, :], xrs_ps[:, :])
                nc.vector.tensor_scalar(
                    out=negxrs[:, :],
                    in0=xrs_ps[:, :],
                    scalar1=-1.0,
                    scalar2=None,
                    op0=alu.mult,
                )

            # ---- rotating group buffers with static channel content
            gbufs = []
            for k in range(NBUF):
                buf = gpool.tile([H, G * FP], f32, tag=f"gbuf{k}")
                v4 = buf[:, :].rearrange("p (g w c) -> p g w c", g=G, c=C)
                # c0 = stamp for every group slot
                nc.gpsimd.tensor_copy(
                    v4[:, :, :, 0],
                    stamp[:, None, :].broadcast_to([H, G, W]),
                )
                # c4 gaps = 0 (bars live at w = 17 + 3i)
                nc.gpsimd.memset(v4[:, :, 0:17, 4], 0.0)
                nc.gpsimd.memset(v4[:, :, 18:110:3, 4], 0.0)
                nc.gpsimd.memset(v4[:, :, 19:111:3, 4], 0.0)
                nc.gpsimd.memset(v4[:, :, 111:128, 4], 0.0)
                gbufs.append(buf)

            with (
                tc.tile_pool(name="p12", bufs=4, space="PSUM") as p12pool,
                tc.tile_pool(name="pbc", bufs=4, space="PSUM") as pbcpool,
            ):
                for g in range(NGROUPS):
                    buf = gbufs[g % NBUF]
                    for j in range(G):
                        b = g * G + j
                        plane = buf[:, j * FP : (j + 1) * FP]
                        vc = plane.rearrange("p (w c) -> p w c", c=C)

                        # scratch2 = scatC * x[b, :] (per-partition scalar)
                        scr = wpool.tile([D, W], f32, tag="scr")
                        nc.gpsimd.tensor_scalar(
                            out=scr[:, :],
                            in0=scatC,
                            scalar1=x_t[:, b : b + 1],
                            scalar2=None,
                            op0=alu.mult,
                        )
                        # c1 scatter + c2 rowcopy into one PSUM tile
                        p12 = p12pool.tile([H, 2 * W], f32, tag="p12")
                        nc.tensor.matmul(p12[:, 0:W], scatR, scr[:, :])
                        nc.tensor.matmul(
                            p12[:, W : 2 * W],
                            onehotR,
                            x_t[:, b : b + 1].broadcast_to([D, W]),
                        )
                        # partition-broadcast of [xs|bh|-xs] row b via basis col
                        pbc = pbcpool.tile([H, 3 * D], f32, tag="pbc")
                        nc.tensor.matmul(
                            pbc[:, :],
                            ident[:, b : b + 1].broadcast_to([BPC, H]),
                            xbh[:, :],
                        )
                        # c3 = |xs_col - xs_row| as max(relu(d), -d)
                        # (abs_max is not encodable on this DVE table)
                        c3v = vc[:, :, 3].rearrange("p (d r) -> p d r", r=4)
                        nc.vector.tensor_scalar(
                            out=c3v,
                            in0=pbc[:, 0:D].unsqueeze(2).broadcast_to([H, D, 4]),
                            scalar1=xrs[:, b : b + 1],
                            scalar2=0.0,
                            op0=alu.subtract,
                            op1=alu.max,
                        )
                        nc.vector.scalar_tensor_tensor(
                            out=c3v,
                            in0=pbc[:, 2 * D : 3 * D]
                            .unsqueeze(2)
                            .broadcast_to([H, D, 4]),
                            scalar=negxrs[:, b : b + 1],
                            in1=c3v,
                            op0=alu.subtract,
                            op1=alu.max,
                        )
                        # c4 bars = (bh > iota)
                        nc.vector.tensor_scalar(
                            out=vc[:, 17:111:3, 4],
                            in0=pbc[:, D : 2 * D],
                            scalar1=iota,
                            scalar2=None,
                            op0=alu.is_gt,
                        )
                        # c1, c2 <- p12 (interleave pairs)
                        nc.scalar.activation(
                            vc[:, :, 1:3],
                            p12[:, :].rearrange("p (c w) -> p w c", c=2),
                            act.Copy,
                        )
                    nc.sync.dma_start(
                        out=out_d[g * G : (g + 1) * G, :, :].rearrange(
                            "b h f -> h b f"
                        ),
                        in_=buf[:, :].rearrange("p (g f) -> p g f", g=G),
                    )
    nc.finalize()
    return nc


def _host_inputs(inputs, stamp, coords):
    """Build the 8 per-core input maps."""
    x = np.ascontiguousarray(inputs, dtype=np.float32)
    stamp2d = np.ascontiguousarray(stamp.reshape(H, W), dtype=np.float32)
    coords = np.asarray(coords)

    scatR = np.zeros((D, W), np.float32)
    scatC = np.zeros((D, W), np.float32)
    scatR[np.arange(D), coords[:, 0]] = 1.0
    scatC[np.arange(D), coords[:, 1]] = 1.0
    # onehotR[d, h] = 1 where row_idx[h] == d
    row_idx = np.repeat(np.arange(D), H // D)
    onehotR = np.zeros((D, H), np.float32)
    onehotR[row_idx, np.arange(H)] = 1.0
    consts32 = np.ascontiguousarray(
        np.concatenate([scatR, scatC, onehotR], axis=1), np.float32
    )
    ident64 = np.eye(BPC, dtype=np.float32)
    misc = np.zeros((H, 2), np.float32)
    misc[:, 0] = np.arange(H)
    misc[:, 1] = 1.0

    maps = []
    for m in range(NCORES):
        xs = x[m * BPC : (m + 1) * BPC]
        maps.append(
            {
                "x_bm": np.ascontiguousarray(xs),
                "x_t": np.ascontiguousarray(xs.T),
                "stamp2d": stamp2d,
                "consts32": consts32,
                "ident64": ident64,
                "misc": misc,
            }
        )
    return maps


class _Runner:
    """Builds the Bass program once and caches the jitted SPMD executable."""

    def __init__(self):
        self.nc = _build_nc()
        self._sharded = None
        self._meta = None

    def _build_exec(self):
        import jax
        import numpy as np
        import concourse.mybir as mybir
        from concourse import bass2jax
        from jax.sharding import Mesh, PartitionSpec
        from jax.experimental.shard_map import shard_map

        bass2jax.install_neuronx_cc_hook()
        nc = self.nc
        partition_name = (
            nc.partition_id_tensor.name if nc.partition_id_tensor else None
        )
        in_names, out_names, out_avals, zero_shapes = [], [], [], []
        for alloc in nc.m.functions[0].allocations:
            if not isinstance(alloc, mybir.MemoryLocationSet):
                continue
            name = alloc.memorylocations[0].name
            if alloc.kind == "ExternalInput":
                if name != partition_name:
                    in_names.append(name)
            elif alloc.kind == "ExternalOutput":
                shape = tuple(alloc.tensor_shape)
                dtype = mybir.dt.np(alloc.dtype)
                out_names.append(name)
                out_avals.append(jax.core.ShapedArray(shape, dtype))
                zero_shapes.append((shape, dtype))
        n_params = len(in_names)
        all_names = in_names + out_names
        if partition_name is not None:
            all_names = all_names + [partition_name]
        donate = tuple(range(n_params, n_params + len(out_names)))

        def _body(*args):
            operands = list(args)
            if partition_name is not None:
                operands.append(bass2jax.partition_id_tensor())
            outs = bass2jax._bass_exec_p.bind(
                *operands,
                out_avals=tuple(out_avals),
                in_names=tuple(all_names),
                out_names=tuple(out_names),
                lowering_input_output_aliases=(),
                sim_require_finite=True,
                sim_require_nnan=True,
                nc=nc,
            )
            return tuple(outs)

        devices = jax.devices()[:NCORES]
        mesh = Mesh(np.asarray(devices), ("core",))
        in_specs = (PartitionSpec("core"),) * (n_params + len(out_names))
        out_specs = (PartitionSpec("core"),) * len(out_names)
        sharded = jax.jit(
            shard_map(
                _body,
                mesh=mesh,
                in_specs=in_specs,
                out_specs=out_specs,
                check_rep=False,
            ),
            donate_argnums=donate,
            keep_unused=True,
        )

        # Output buffers are donated bass_exec operands; build them on
        # device (sharded memset) instead of shipping 168MB of host zeros
        # through axon every call.
        import jax.numpy as jnp
        from jax.sharding import NamedSharding

        shardings = tuple(
            NamedSharding(mesh, PartitionSpec("core")) for _ in zero_shapes
        )

        def _make_zeros():
            return tuple(
                jnp.zeros((NCORES * s[0], *s[1:]), dt) for (s, dt) in zero_shapes
            )

        self._zeros_fn = jax.jit(_make_zeros, out_shardings=shardings)
        self._sharded = sharded
        self._meta = (in_names, out_names, zero_shapes)

    def run(self, in_maps):
        if self._sharded is None:
            self._build_exec()
        in_names, out_names, zero_shapes = self._meta
        concat_in = [
            np.concatenate([np.asarray(m[name]) for m in in_maps], axis=0)
            for name in in_names
        ]
        out_arrs = self._sharded(*concat_in, *self._zeros_fn())
        outs = [np.asarray(a) for a in out_arrs]
        per_core = []
        for c in range(NCORES):
            per_core.append(
                {
                    name: outs[i].reshape(NCORES, *zero_shapes[i][0])[c]
                    for i, name in enumerate(out_names)
                }
            )
        return per_core


def _get_runner():
    global _RUNNER
    if _RUNNER is None:
        _RUNNER = _Runner()
    return _RUNNER


def kernel(inputs, stamp, coords):
    inputs = np.asarray(inputs)
    stamp = np.asarray(stamp)
    coords = np.asarray(coords)
    runner = _get_runner()
    in_maps = _host_inputs(inputs, stamp, coords)
    results = runner.run(in_maps)
    out = np.stack([r["out"] for r in results], axis=0)  # [8, 64, H, W*C]
    out = out.reshape(B, H, W, C).astype(np.float32)
    return out


# revision 18
# speedup vs baseline: 26.9271x; 1.0281x over previous
"""DeepInsight encoding kernel for 8 Trainium2 NeuronCores.

Data-parallel over batch: each core builds 64 interleaved [H, W*5] output
planes in SBUF and streams them to HBM as large contiguous DMAs.

Channels per output plane [h, w, c]:
  c0: stamp (static, written once per rotating buffer)
  c1: scatter-add of x at coords (PE matmul with host-built one-hots)
  c2: row-wise copy x[row_idx[h]] (PE matmul, broadcast along w)
  c3: |x_i - x_j| / (max-min) upsampled 4x4 (DVE subtract+abs_max)
  c4: equidistant bars y < round(128*x) (DVE is_gt vs iota), gaps static 0
"""

import numpy as np

B, D, H, W, C = 512, 32, 128, 128, 5
NCORES = 8
BPC = B // NCORES            # 64 batches per core
G = 8                        # max batches per output DMA group
# Small leading groups let the first output DMA start early (the output
# stream is otherwise perfectly back-to-back and ramp dominates waste).
GROUP_SIZES = [2, 2, 4, 4, 4] + [8] * 6
assert sum(GROUP_SIZES) == BPC
NBUF = 4                     # rotating SBUF plane buffers
FP = W * C                   # 640 floats per output row
MAGIC = float(2.0 ** 23)     # fp32 round-to-nearest-even trick

# packed input blob layout (all f32, [128, BLOB_W]); the first-DMA slice
# carries everything on the c3/c4 critical path (x, onehotR, ident)
_XBM0 = 0              # [64, 32]
_XT0 = 32              # [32, 64]
_IOTA0 = 96            # [128, 1]
_ONES0 = 97            # [128, 1] (only row 0 used)
_ONEHOTR0 = 98         # [32, 128]
_IDENT0 = 226          # [64, 64]
_XPART = 290           # end of the early DMA slice
_STAMP0 = 290          # [128, 128]
_SCATR0 = 418          # [32, 128]
_SCATC0 = 546          # [32, 128]
BLOB_W = 674

_RUNNER = None


def _build_nc():
    import concourse.bacc as bacc
    import concourse.mybir as mybir
    from concourse.tile import TileContext

    f32 = mybir.dt.float32
    alu = mybir.AluOpType
    act = mybir.ActivationFunctionType

    # Bacc (not raw Bass): its finalize() pipeline runs
    # move_matmul_waits_to_ldweights + generate_event_semaphores, which
    # split sync-waits to satisfy TRN2's 1-wait-per-instruction limit.
    nc = bacc.Bacc()
    blob_d = nc.dram_tensor("blob", [H, BLOB_W], f32, kind="ExternalInput")
    out_d = nc.dram_tensor("out", [BPC, H, FP], f32, kind="ExternalOutput")

    with TileContext(nc) as tc:
        with (
            tc.tile_pool(name="const", bufs=1) as cpool,
            tc.tile_pool(name="gbuf", bufs=1) as gpool,
            tc.tile_pool(name="work", bufs=4) as wpool,
            tc.tile_pool(name="small", bufs=1) as spool,
        ):
            # ---- load inputs: critical-path columns first, then the rest
            blob = cpool.tile([H, BLOB_W], f32, tag="blob")
            nc.sync.dma_start(
                out=blob[:, 0:_XPART], in_=blob_d[:, 0:_XPART]
            )
            nc.sync.dma_start(
                out=blob[:, _XPART:BLOB_W], in_=blob_d[:, _XPART:BLOB_W]
            )
            stamp = blob[:, _STAMP0 : _STAMP0 + W]
            iota = blob[:, _IOTA0 : _IOTA0 + 1]
            ones_cell = blob[0:1, _ONES0 : _ONES0 + 1]
            x_bm = blob[0:BPC, _XBM0 : _XBM0 + D]
            x_t = blob[0:D, _XT0 : _XT0 + BPC]
            scatR = blob[0:D, _SCATR0 : _SCATR0 + W]
            scatC = blob[0:D, _SCATC0 : _SCATC0 + W]
            onehotR = blob[0:D, _ONEHOTR0 : _ONEHOTR0 + W]
            ident = blob[0:BPC, _IDENT0 : _IDENT0 + BPC]

            # ---- setup
            inv_sb = spool.tile([BPC, 4], f32, tag="inv")  # min|max|r|invr
            xbh = spool.tile([BPC, 2 * D], f32, tag="xbh")  # [x | bh]
            xr = spool.tile([H, BPC], f32, tag="xr")        # x at row_idx[h]
            invr_bc = spool.tile([H, BPC], f32, tag="invr_bc")
            invr_row = spool.tile([1, BPC], f32, tag="invr_row")

            # x copy into the pbc rhs tile
            nc.vector.tensor_copy(xbh[:, 0:D], x_bm)
            # bh = round_half_even(128 * x): (x*128 + 2^23) - 2^23
            nc.vector.tensor_scalar(
                out=xbh[:, D : 2 * D],
                in0=x_bm,
                scalar1=128.0,
                scalar2=MAGIC,
                op0=alu.mult,
                op1=alu.add,
            )
            nc.vector.tensor_scalar(
                out=xbh[:, D : 2 * D],
                in0=xbh[:, D : 2 * D],
                scalar1=MAGIC,
                scalar2=None,
                op0=alu.subtract,
            )
            # invr = 1 / (max - min)
            nc.vector.tensor_reduce(
                inv_sb[:, 0:1], x_bm, axis=mybir.AxisListType.X, op=alu.min
            )
            nc.vector.tensor_reduce(
                inv_sb[:, 1:2], x_bm, axis=mybir.AxisListType.X, op=alu.max
            )
            nc.vector.tensor_tensor(
                out=inv_sb[:, 2:3],
                in0=inv_sb[:, 1:2],
                in1=inv_sb[:, 0:1],
                op=alu.subtract,
            )
            nc.vector.reciprocal(inv_sb[:, 3:4], inv_sb[:, 2:3])

            with tc.tile_pool(name="psetup", bufs=2, space="PSUM") as psetup:
                # xr [128, 64] = onehotR.T @ x_t (unscaled row gather)
                xr_ps = psetup.tile([H, BPC], f32, tag="xr_ps")
                nc.tensor.matmul(xr_ps[:, :], onehotR, x_t)
                nc.vector.tensor_copy(xr[:, :], xr_ps[:, :])

                # invr_bc [128, 64]: invr broadcast down all partitions
                # (PE transpose to a row, then K=1 ones matmul)
                invr_row_ps = psetup.tile([1, BPC], f32, tag="invr_row_ps")
                nc.tensor.transpose(invr_row_ps[:, :], inv_sb[:, 3:4], ident)
                nc.vector.tensor_copy(invr_row[:, :], invr_row_ps[:, :])
                invr_bc_ps = psetup.tile([H, BPC], f32, tag="invr_bc_ps")
                nc.tensor.matmul(
                    invr_bc_ps[:, :],
                    ones_cell.broadcast_to([1, H]),
                    invr_row[:, :],
                )
                nc.vector.tensor_copy(invr_bc[:, :], invr_bc_ps[:, :])

            # ---- rotating group buffers (static content filled per group)
            gbufs = []
            for k in range(NBUF):
                gb = gpool.tile([H, G * FP], f32, tag=f"gbuf{k}")
                gbufs.append(gb)

            with (
                tc.tile_pool(name="p12", bufs=4, space="PSUM") as p12pool,
                tc.tile_pool(name="pbc", bufs=4, space="PSUM") as pbcpool,
            ):
                base = 0
                for g, gs in enumerate(GROUP_SIZES):
                    buf = gbufs[g % NBUF]
                    # static channels for exactly this group's slots:
                    # c0 = stamp; c4 gaps = 0 (bars live at w = 17 + 3i)
                    v4 = buf[:, 0 : gs * FP].rearrange(
                        "p (g w c) -> p g w c", g=gs, c=C
                    )
                    nc.gpsimd.tensor_copy(
                        v4[:, :, :, 0],
                        stamp.unsqueeze(1).broadcast_to([H, gs, W]),
                    )
                    nc.gpsimd.memset(v4[:, :, 0:17, 4], 0.0)
                    nc.gpsimd.memset(v4[:, :, 18:110:3, 4], 0.0)
                    nc.gpsimd.memset(v4[:, :, 19:111:3, 4], 0.0)
                    nc.gpsimd.memset(v4[:, :, 111:128, 4], 0.0)
                    for j in range(gs):
                        b = base + j
                        plane = buf[:, j * FP : (j + 1) * FP]
                        vc = plane.rearrange("p (w c) -> p w c", c=C)

                        # partition-broadcast of [x | bh] row b (basis col)
                        pbc = pbcpool.tile([H, 2 * D], f32, tag="pbc")
                        nc.tensor.matmul(
                            pbc[:, :],
                            ident[:, b : b + 1].broadcast_to([BPC, H]),
                            xbh[:, :],
                        )
                        # c3 raw diff d = x_col - x_row (4x repeat along w)
                        c3s = wpool.tile([H, W], f32, tag="c3s")
                        nc.vector.tensor_scalar(
                            out=c3s[:, :].rearrange("p (d r) -> p d r", r=4),
                            in0=pbc[:, 0:D].unsqueeze(2).broadcast_to([H, D, 4]),
                            scalar1=xr[:, b : b + 1],
                            scalar2=None,
                            op0=alu.subtract,
                        )
                        # c4 bars = (bh > iota)
                        nc.vector.tensor_scalar(
                            out=vc[:, 17:111:3, 4],
                            in0=pbc[:, D : 2 * D],
                            scalar1=iota,
                            scalar2=None,
                            op0=alu.is_gt,
                        )
                        # c3 = |invr * d| on ACT (scale is per-partition)
                        nc.scalar.activation(
                            vc[:, :, 3].rearrange("p (d r) -> p d r", r=4),
                            c3s[:, :].rearrange("p (d r) -> p d r", r=4),
                            act.Abs,
                            scale=invr_bc[:, b : b + 1],
                        )
                        # scratch = scatC * x[b, :] (per-partition scalar)
                        scr = wpool.tile([D, W], f32, tag="scr")
                        nc.gpsimd.tensor_scalar(
                            out=scr[:, :],
                            in0=scatC,
                            scalar1=x_t[:, b : b + 1],
                            scalar2=None,
                            op0=alu.mult,
                        )
                        # c1 scatter + c2 rowcopy into one PSUM tile
                        p12 = p12pool.tile([H, 2 * W], f32, tag="p12")
                        nc.tensor.matmul(p12[:, 0:W], scatR, scr[:, :])
                        nc.tensor.matmul(
                            p12[:, W : 2 * W],
                            onehotR,
                            x_t[:, b : b + 1].broadcast_to([D, W]),
                        )
                        # c1, c2 <- p12 (interleave pairs); alternate the
                        # engine so ACT and DVE stay balanced
                        if b % 2 == 0:
                            nc.scalar.activation(
                                vc[:, :, 1:3],
                                p12[:, :].rearrange("p (c w) -> p w c", c=2),
                                act.Copy,
                            )
                        else:
                            nc.vector.tensor_copy(
                                vc[:, :, 1:3],
                                p12[:, :].rearrange("p (c w) -> p w c", c=2),
                            )
                    nc.sync.dma_start(
                        out=out_d[base : base + gs, :, :].rearrange(
                            "b h f -> h b f"
                        ),
                        in_=buf[:, 0 : gs * FP].rearrange(
                            "p (g f) -> p g f", g=gs
                        ),
                    )
                    base += gs
    nc.finalize()
    return nc


def _host_inputs(inputs, stamp, coords):
    """Build the 8 per-core input maps (one packed blob each)."""
    x = np.ascontiguousarray(inputs, dtype=np.float32)
    stamp2d = np.ascontiguousarray(stamp.reshape(H, W), dtype=np.float32)
    coords = np.asarray(coords)

    base = np.zeros((H, BLOB_W), np.float32)
    base[:, _STAMP0 : _STAMP0 + W] = stamp2d
    base[:, _IOTA0] = np.arange(H)
    base[0, _ONES0] = 1.0  # (layout: x cols 0:96 filled per core below)
    scatR = np.zeros((D, W), np.float32)
    scatC = np.zeros((D, W), np.float32)
    scatR[np.arange(D), coords[:, 0]] = 1.0
    scatC[np.arange(D), coords[:, 1]] = 1.0
    row_idx = np.repeat(np.arange(D), H // D)
    onehotR = np.zeros((D, H), np.float32)
    onehotR[row_idx, np.arange(H)] = 1.0
    base[0:D, _SCATR0 : _SCATR0 + W] = scatR
    base[0:D, _SCATC0 : _SCATC0 + W] = scatC
    base[0:D, _ONEHOTR0 : _ONEHOTR0 + W] = onehotR
    base[0:BPC, _IDENT0 : _IDENT0 + BPC] = np.eye(BPC, dtype=np.float32)

    maps = []
    for m in range(NCORES):
        xs = x[m * BPC : (m + 1) * BPC]
        blob = base.copy()
        blob[0:BPC, _XBM0 : _XBM0 + D] = xs
        blob[0:D, _XT0 : _XT0 + BPC] = xs.T
        maps.append({"blob": blob})
    return maps


class _Runner:
    """Builds the Bass program once and caches the jitted SPMD executable."""

    def __init__(self):
        self.nc = _build_nc()
        self._sharded = None
        self._meta = None

    def _build_exec(self):
        import jax
        import numpy as np
        import concourse.mybir as mybir
        from concourse import bass2jax
        from jax.sharding import Mesh, PartitionSpec
        from jax.experimental.shard_map import shard_map

        bass2jax.install_neuronx_cc_hook()
        nc = self.nc
        partition_name = (
            nc.partition_id_tensor.name if nc.partition_id_tensor else None
        )
        in_names, out_names, out_avals, zero_shapes = [], [], [], []
        for alloc in nc.m.functions[0].allocations:
            if not isinstance(alloc, mybir.MemoryLocationSet):
                continue
            name = alloc.memorylocations[0].name
            if alloc.kind == "ExternalInput":
                if name != partition_name:
                    in_names.append(name)
            elif alloc.kind == "ExternalOutput":
                shape = tuple(alloc.tensor_shape)
                dtype = mybir.dt.np(alloc.dtype)
                out_names.append(name)
                out_avals.append(jax.core.ShapedArray(shape, dtype))
                zero_shapes.append((shape, dtype))
        n_params = len(in_names)
        all_names = in_names + out_names
        if partition_name is not None:
            all_names = all_names + [partition_name]
        donate = tuple(range(n_params, n_params + len(out_names)))

        def _body(*args):
            operands = list(args)
            if partition_name is not None:
                operands.append(bass2jax.partition_id_tensor())
            outs = bass2jax._bass_exec_p.bind(
                *operands,
                out_avals=tuple(out_avals),
                in_names=tuple(all_names),
                out_names=tuple(out_names),
                lowering_input_output_aliases=(),
                sim_require_finite=True,
                sim_require_nnan=True,
                nc=nc,
            )
            return tuple(outs)

        devices = jax.devices()[:NCORES]
        mesh = Mesh(np.asarray(devices), ("core",))
        in_specs = (PartitionSpec("core"),) * (n_params + len(out_names))
        out_specs = (PartitionSpec("core"),) * len(out_names)
        sharded = jax.jit(
            shard_map(
                _body,
                mesh=mesh,
                in_specs=in_specs,
                out_specs=out_specs,
                check_rep=False,
            ),
            donate_argnums=donate,
            keep_unused=True,
        )

        # Output buffers are donated bass_exec operands; build them on
        # device (sharded memset) instead of shipping 168MB of host zeros
        # through axon every call.
        import jax.numpy as jnp
        from jax.sharding import NamedSharding

        shardings = tuple(
            NamedSharding(mesh, PartitionSpec("core")) for _ in zero_shapes
        )

        def _make_zeros():
            return tuple(
                jnp.zeros((NCORES * s[0], *s[1:]), dt) for (s, dt) in zero_shapes
            )

        self._zeros_fn = jax.jit(_make_zeros, out_shardings=shardings)
        self._sharded = sharded
        self._meta = (in_names, out_names, zero_shapes)

    def run(self, in_maps):
        if self._sharded is None:
            self._build_exec()
        in_names, out_names, zero_shapes = self._meta
        concat_in = [
            np.concatenate([np.asarray(m[name]) for m in in_maps], axis=0)
            for name in in_names
        ]
        out_arrs = self._sharded(*concat_in, *self._zeros_fn())
        outs = [np.asarray(a) for a in out_arrs]
        per_core = []
        for c in range(NCORES):
            per_core.append(
                {
                    name: outs[i].reshape(NCORES, *zero_shapes[i][0])[c]
                    for i, name in enumerate(out_names)
                }
            )
        return per_core


def _get_runner():
    global _RUNNER
    if _RUNNER is None:
        _RUNNER = _Runner()
    return _RUNNER


def kernel(inputs, stamp, coords):
    inputs = np.asarray(inputs)
    stamp = np.asarray(stamp)
    coords = np.asarray(coords)
    runner = _get_runner()
    in_maps = _host_inputs(inputs, stamp, coords)
    results = runner.run(in_maps)
    out = np.stack([r["out"] for r in results], axis=0)  # [8, 64, H, W*C]
    out = out.reshape(B, H, W, C).astype(np.float32)
    return out
